# revision 24
# baseline (speedup 1.0000x reference)
"""Trainium2 Bass kernel for KnowledgeEmbeddings (ragged_sequence).

Contract: kernel(**inputs) takes FULL unsharded inputs (numpy), returns the
FULL [64, 320, 768] f32 output.  Internally shards batch rows over 8
NeuronCores (8 rows each), replicates embedding tables, and runs a Tile/Bass
kernel per core via run_bass_kernel_spmd.

V9: device IO bf16.  tt/pos table applied via one-hot matmuls (Tensor) from
an SBUF-resident 384-row table; per-tile [128,768] PSUM with (512,256)
N-splits; word merge+row-sum fused in one DVE scalar_tensor_tensor (accum
gives SM); SS via ACT Square; xhat via ACT Identity(scale=rstd, bias=-u*rstd)
on word tiles / DVE tensor_scalar on kn tiles; gamma/beta applied pair-wise
from a host-replicated broadcast tile; OH loads triggered from the Tensor
queue and output writes from the Vector queue to unload the Sync sequencer.
"""

import functools
import numpy as np
import ml_dtypes

import concourse.bass as bass
import concourse.tile as tile
from concourse import bacc, mybir
from concourse.bass import IndirectOffsetOnAxis
from concourse.bass_utils import run_bass_kernel_spmd

# Problem constants (hardcoded per spec nn_KnowledgeEmbeddings_80839874445880)
WORD_LEN = 256
KN_LEN = 64
VOCAB = 30522
N_ENT = 500000
HID = 768
MAX_POS = 512
N_TYPES = 2
D_ENT = 100
B = 64
SEQ = WORD_LEN + KN_LEN  # 320
EPS = 1e-12

NCORES = 8
ROWS = B // NCORES           # 8 batch rows per core
WT = ROWS * WORD_LEN // 128  # 16 word tiles per core
KT = ROWS * KN_LEN // 128    # 4 knowledge tiles per core
NT = WT + KT                 # 20 tiles per core
SGRP = 8                     # word tiles per stats group
TROWS = 384                  # padded tt/pos table rows (3 chunks of 128)
NCHUNK = 3
NSPLITS = ((0, 512), (512, 256))  # matmul N-splits within a PSUM bank pair

f32 = mybir.dt.float32
bf16 = mybir.dt.bfloat16
i32 = mybir.dt.int32
AF = mybir.ActivationFunctionType
ALU = mybir.AluOpType
BF = ml_dtypes.bfloat16


# ---------------------------------------------------------------- host side

def _compact(ids: np.ndarray, tts: np.ndarray):
    """Vectorized numpy mirror of reference._compact_row."""
    ids = ids.astype(np.int64)
    wmask = (ids > 0) & (ids < VOCAB)
    worder = np.argsort(~wmask, axis=1, kind="stable")[:, :WORD_LEN]
    nw = wmask.sum(1, keepdims=True)
    wvalid = np.arange(WORD_LEN)[None, :] < nw
    wid = np.where(wvalid, np.take_along_axis(ids, worder, 1), 0)
    wtt = np.where(wvalid, np.take_along_axis(tts, worder, 1), 1)
    wpos = np.where(wvalid, worder, np.arange(WORD_LEN)[None, :])

    kmask = ids >= VOCAB
    korder = np.argsort(~kmask, axis=1, kind="stable")[:, :KN_LEN]
    nk = kmask.sum(1, keepdims=True)
    kvalid = np.arange(KN_LEN)[None, :] < nk
    kid = np.where(kvalid, np.take_along_axis(ids, korder, 1) - VOCAB, 0)
    ktt = np.where(kvalid, np.take_along_axis(tts, korder, 1), 0)
    kpos = np.where(kvalid, korder, 0)
    return wid, wtt, wpos, kid, ktt, kpos, kvalid


# ------------------------------------------------------------- device side

def _gather(nc, out_ap, table_ap, idx_ap):
    nc.gpsimd.indirect_dma_start(
        out=out_ap, out_offset=None, in_=table_ap,
        in_offset=IndirectOffsetOnAxis(ap=idx_ap, axis=0),
    )


def _finish_stats(nc, spool, eps_sb, SS, SM, n, tag, kv=None):
    """Batched [128, n] stat math (f32).  Returns (NUR, RSTD).

    U = -SM/HID;  RSTD = 1/sqrt(SS/HID - U^2 + eps) (times kv if given);
    NUR = U*RSTD  so that  xhat = X*RSTD + NUR.
    """
    U_t = spool.tile([128, n], f32, tag=f"U{tag}")
    U = U_t[:, :n]
    nc.scalar.mul(U, SM, -1.0 / HID)
    SSs_t = spool.tile([128, n], f32, tag=f"SSs{tag}")
    SSs = SSs_t[:, :n]
    nc.scalar.mul(SSs, SS, 1.0 / HID)
    USQ_t = spool.tile([128, n], f32, tag=f"USQ{tag}")
    USQ = USQ_t[:, :n]
    nc.vector.tensor_mul(USQ, U, U)
    VAR_t = spool.tile([128, n], f32, tag=f"VAR{tag}")
    VAR = VAR_t[:, :n]
    nc.vector.tensor_tensor(out=VAR, in0=SSs, in1=USQ, op=ALU.subtract)
    RSTD_t = spool.tile([128, n], f32, tag=f"RSTD{tag}")
    RSTD = RSTD_t[:, :n]
    nc.scalar.activation(RSTD, VAR, func=AF.Sqrt, bias=eps_sb[:])
    nc.vector.reciprocal(RSTD, RSTD)
    if kv is not None:
        nc.vector.tensor_mul(RSTD, RSTD, kv)
    NUR_t = spool.tile([128, n], f32, tag=f"NUR{tag}")
    NUR = NUR_t[:, :n]
    nc.vector.tensor_mul(NUR, U, RSTD)
    return NUR, RSTD


def _device_kernel(tc, aps):
    nc = tc.nc
    we, ev, postab, kw, gbb2, oh, idx, kvf, out = (
        aps["word_emb"], aps["entity_vec"], aps["postab"], aps["kw"],
        aps["gbb2"], aps["oh"], aps["idx"], aps["kvalid"], aps["out"],
    )
    import contextlib
    with contextlib.ExitStack() as ctx:
        singles = ctx.enter_context(tc.tile_pool(name="singles", bufs=1))
        ohpool = ctx.enter_context(tc.tile_pool(name="oh", bufs=1))
        spool = ctx.enter_context(tc.tile_pool(name="small", bufs=2))
        scrpool = ctx.enter_context(tc.tile_pool(name="scr", bufs=2))
        evtpool = ctx.enter_context(tc.tile_pool(name="evt", bufs=2))
        mmpsum = ctx.enter_context(tc.tile_pool(name="mm", bufs=3, space="PSUM"))
        trpsum = ctx.enter_context(tc.tile_pool(name="tr", bufs=2, space="PSUM"))

        # --- index loads, then indirect gathers as early as possible
        idx_sb = singles.tile([128, NT], i32)
        nc.sync.dma_start(idx_sb[:], idx)
        kv_sb = singles.tile([128, KT], f32)
        nc.sync.dma_start(kv_sb[:], kvf)

        # group X tiles: word grp0, word grp1, kn (pair-contiguous free dim)
        XG = [singles.tile([128, SGRP * HID], bf16, name=f"XG{g}",
                           tag=f"XG{g}")
              for g in range(2)]
        XK = singles.tile([128, KT * HID], bf16)
        EV = singles.tile([128, KT * D_ENT], bf16)

        def xw(t):
            g, i = divmod(t, SGRP)
            return XG[g][:, HID * i:HID * (i + 1)]

        # --- preload ALL static inputs before gathers: DMA-completion
        # semaphores are cumulative per queue, so anything issued after the
        # gathers would wait for them.
        postab_sb = singles.tile([128, NCHUNK * HID], bf16)
        nc.sync.dma_start(postab_sb[:], postab)
        ident = singles.tile([128, 128], bf16)
        nc.sync.dma_start(ident[:], aps["ident"])
        kw_sb = singles.tile([128, HID], bf16)
        nc.sync.dma_start(kw_sb[:], kw)
        oh_sbs = []
        for t in range(NT):
            oh_sb = ohpool.tile([128, NCHUNK * 128], bf16, name=f"oh{t}",
                                tag=f"oh{t}")
            nc.sync.dma_start(oh_sb[:], oh[128 * t:128 * (t + 1), :])
            oh_sbs.append(oh_sb)
        gbb = singles.tile([128, 4 * 2 * HID], bf16)  # [gw, bw, gk, bk] x1536
        nc.sync.dma_start(gbb[:], gbb2)
        eps_sb = singles.tile([128, 1], f32)
        nc.vector.memset(eps_sb[:], EPS)

        for t in range(WT):
            _gather(nc, xw(t), we, idx_sb[:, t:t + 1])
        for c in range(KT):
            _gather(nc, EV[:, D_ENT * c:D_ENT * (c + 1)], ev,
                    idx_sb[:, WT + c:WT + c + 1])
        # small (1.5MB) kn gamma/beta x4 tile: after the gathers in the DMA
        # FIFO (lands ~41us, first use ~52us)
        gbbk = singles.tile([128, 2 * KT * HID], bf16)
        nc.sync.dma_start(gbbk[:], aps["gbbk"])

        def table_matmuls(pm, oh_sb, extra=None):
            for (o, n) in NSPLITS:
                for c in range(NCHUNK):
                    nc.tensor.matmul(
                        out=pm[:, o:o + n], lhsT=oh_sb[:, 128 * c:128 * (c + 1)],
                        rhs=postab_sb[:, HID * c + o:HID * c + o + n],
                        start=(c == 0), stop=(c == NCHUNK - 1 and extra is None),
                    )
                if extra is not None:
                    lhsT, rhs = extra
                    nc.tensor.matmul(out=pm[:, o:o + n], lhsT=lhsT,
                                     rhs=rhs[:, o:o + n], start=False, stop=True)

        def word_produce(grp):
            SS = spool.tile([128, SGRP], f32, tag=f"SS{grp}")
            SM = spool.tile([128, SGRP], f32, tag=f"SM{grp}")
            for i in range(SGRP):
                t = grp * SGRP + i
                X = xw(t)
                oh_sb = oh_sbs[t]
                pm = mmpsum.tile([128, HID], f32, tag="mm")
                table_matmuls(pm, oh_sb)
                # X += pm, SM = row-sum(X)  in one DVE op
                nc.vector.scalar_tensor_tensor(
                    out=X, in0=X, scalar=0.0, in1=pm[:],
                    op0=ALU.add, op1=ALU.add, accum_out=SM[:, i:i + 1])
                scr = scrpool.tile([128, HID], bf16, tag="scr")
                nc.scalar.activation(scr[:], X, func=AF.Square,
                                     accum_out=SS[:, i:i + 1])
            return SS, SM

        def word_stats_xhat(grp, SS, SM):
            NUR, RSTD = _finish_stats(nc, spool, eps_sb, SS[:], SM[:], SGRP,
                                      f"w{grp}")
            for i in range(SGRP):
                t = grp * SGRP + i
                X = xw(t)
                nc.vector.tensor_scalar(
                    out=X, in0=X, scalar1=RSTD[:, i:i + 1],
                    scalar2=NUR[:, i:i + 1], op0=ALU.mult, op1=ALU.add)

        def word_gb_write(grp):
            for p in range(SGRP // 2):
                Xp = XG[grp][:, 2 * HID * p:2 * HID * (p + 1)]
                nc.vector.tensor_mul(Xp, Xp, gbb[:, 0:2 * HID])
                nc.vector.tensor_add(Xp, Xp, gbb[:, 2 * HID:4 * HID])
                for i in (2 * p, 2 * p + 1):
                    t = grp * SGRP + i
                    b, h = divmod(t, 2)
                    r = b * SEQ + h * 128
                    nc.sync.dma_start(out[r:r + 128, :], xw(t))

        def kn_tiles():
            SSk = spool.tile([128, KT], f32, tag="SSk")
            SMk = spool.tile([128, KT], f32, tag="SMk")
            for c in range(KT):
                Xc = XK[:, HID * c:HID * (c + 1)]
                oh_sb = oh_sbs[WT + c]
                ps_t = trpsum.tile([D_ENT, 128], bf16, tag="pst")
                nc.tensor.transpose(out=ps_t[:],
                                    in_=EV[:, D_ENT * c:D_ENT * (c + 1)],
                                    identity=ident[:])
                EVT = evtpool.tile([128, 128], bf16, tag="EVT")
                nc.vector.memset(EVT[:], 0.0)
                nc.scalar.copy(EVT[:D_ENT, :], ps_t[:])
                pm = mmpsum.tile([128, HID], f32, tag="mm")
                table_matmuls(pm, oh_sb, extra=(EVT[:], kw_sb))
                nc.scalar.activation(Xc, pm[:], func=AF.Copy,
                                     accum_out=SMk[:, c:c + 1])
                scr = scrpool.tile([128, HID], bf16, tag="scr")
                nc.scalar.activation(scr[:], Xc, func=AF.Square,
                                     accum_out=SSk[:, c:c + 1])
            NUR, RSTD = _finish_stats(nc, spool, eps_sb, SSk[:], SMk[:], KT,
                                      "k", kv=kv_sb[:])
            for c in range(KT):
                Xc = XK[:, HID * c:HID * (c + 1)]
                nc.vector.tensor_scalar(
                    out=Xc, in0=Xc, scalar1=RSTD[:, c:c + 1],
                    scalar2=NUR[:, c:c + 1], op0=ALU.mult, op1=ALU.add)
            nc.vector.tensor_mul(XK[:], XK[:], gbbk[:, :KT * HID])
            nc.vector.tensor_add(XK[:], XK[:], gbbk[:, KT * HID:])
            for c in range(KT):
                r0 = (2 * c) * SEQ + WORD_LEN
                r1 = (2 * c + 1) * SEQ + WORD_LEN
                nc.sync.dma_start(out[r0:r0 + 64, :], XK[0:64, HID * c:HID * (c + 1)])
                nc.sync.dma_start(out[r1:r1 + 64, :], XK[64:128, HID * c:HID * (c + 1)])

        SS0, SM0 = word_produce(0)
        word_stats_xhat(0, SS0, SM0)
        SS1, SM1 = word_produce(1)
        word_gb_write(0)
        word_stats_xhat(1, SS1, SM1)
        word_gb_write(1)
        kn_tiles()


@functools.lru_cache(maxsize=1)
def build_program():
    nc = bacc.Bacc("TRN2", target_bir_lowering=False, debug=False,
                   enable_asserts=False)
    aps = {
        "word_emb": nc.dram_tensor("word_emb", [VOCAB, HID], bf16,
                                   kind="ExternalInput").ap(),
        "entity_vec": nc.dram_tensor("entity_vec", [N_ENT, D_ENT], bf16,
                                     kind="ExternalInput").ap(),
        "postab": nc.dram_tensor("postab", [128, NCHUNK * HID], bf16,
                                 kind="ExternalInput").ap(),
        "kw": nc.dram_tensor("kw", [128, HID], bf16,
                             kind="ExternalInput").ap(),
        "gbb2": nc.dram_tensor("gbb2", [128, 4 * 2 * HID], bf16,
                               kind="ExternalInput").ap(),
        "gbbk": nc.dram_tensor("gbbk", [128, 2 * KT * HID], bf16,
                               kind="ExternalInput").ap(),
        "oh": nc.dram_tensor("oh", [NT * 128, NCHUNK * 128], bf16,
                             kind="ExternalInput").ap(),
        "idx": nc.dram_tensor("idx", [128, NT], i32,
                              kind="ExternalInput").ap(),
        "ident": nc.dram_tensor("ident", [128, 128], bf16,
                                kind="ExternalInput").ap(),
        "kvalid": nc.dram_tensor("kvalid", [128, KT], f32,
                                 kind="ExternalInput").ap(),
        "out": nc.dram_tensor("out", [ROWS * SEQ, HID], bf16,
                              kind="ExternalOutput").ap(),
    }
    with tile.TileContext(nc) as tc:
        _device_kernel(tc, aps)
    nc.compile()
    return nc


def _prepare_in_maps(inputs):
    input_ids = np.asarray(inputs["input_ids"], dtype=np.int32)
    token_type_ids = np.asarray(inputs["token_type_ids"], dtype=np.int32)
    word_emb = np.ascontiguousarray(
        np.asarray(inputs["word_emb"], np.float32).astype(BF))
    pos_emb = np.asarray(inputs["pos_emb"], np.float32)
    tt_emb = np.asarray(inputs["tt_emb"], np.float32)
    entity_vec = np.ascontiguousarray(
        np.asarray(inputs["entityVec"], np.float32).astype(BF))
    ke_w = np.asarray(inputs["ke_w"], np.float32)
    ke_b = np.asarray(inputs["ke_b"], np.float32)

    # tt/pos table: rows 0-319 = pos_emb[p] + tt_emb[0]; row 320 =
    # tt_emb[1]-tt_emb[0] (keyed by token tt in the one-hot); row 321 = ke_b
    # (knowledge tiles only); padded to 384 rows, stored [128, 3, HID].
    ptab = np.zeros((TROWS, HID), np.float32)
    ptab[:SEQ] = pos_emb[:SEQ] + tt_emb[0]
    ptab[SEQ] = tt_emb[1] - tt_emb[0]
    ptab[SEQ + 1] = ke_b
    postab = np.ascontiguousarray(
        ptab.reshape(NCHUNK, 128, HID).transpose(1, 0, 2)
        .reshape(128, NCHUNK * HID).astype(BF))

    kw = np.zeros((128, HID), np.float32)
    kw[:D_ENT] = ke_w.T
    kw = np.ascontiguousarray(kw.astype(BF))

    # pair-wide gamma/beta broadcast rows: [gw gw | bw bw | gk gk | bk bk]
    gbrow = np.concatenate([
        np.tile(np.asarray(inputs["w_gamma"], np.float32), 2),
        np.tile(np.asarray(inputs["w_beta"], np.float32), 2),
        np.tile(np.asarray(inputs["k_gamma"], np.float32), 2),
        np.tile(np.asarray(inputs["k_beta"], np.float32), 2),
    ])
    gbb2 = np.ascontiguousarray(
        np.broadcast_to(gbrow[None, :], (128, 4 * 2 * HID)).astype(BF))
    gbrowk = np.concatenate([
        np.tile(np.asarray(inputs["k_gamma"], np.float32), KT),
        np.tile(np.asarray(inputs["k_beta"], np.float32), KT),
    ])
    gbbk = np.ascontiguousarray(
        np.broadcast_to(gbrowk[None, :], (128, 2 * KT * HID)).astype(BF))

    wid, wtt, wpos, kid, ktt, kpos, kvalid = _compact(input_ids, token_type_ids)
    kvf = kvalid.astype(np.float32)

    m_idx = np.arange(128)[None, :]
    in_maps = []
    for core in range(NCORES):
        s = slice(core * ROWS, (core + 1) * ROWS)
        idx = np.concatenate([
            wid[s].reshape(WT, 128).T.astype(np.int32),
            kid[s].reshape(KT, 128).T.astype(np.int32),
        ], axis=1)

        oh = np.zeros((NT, TROWS, 128), np.float32)
        wp = wpos[s].reshape(WT, 128)
        oh[np.arange(WT)[:, None], wp, m_idx] = 1.0
        oh[:WT, SEQ, :] = wtt[s].reshape(WT, 128)
        kp = kpos[s].reshape(KT, 128)
        oh[WT + np.arange(KT)[:, None], kp, m_idx] = 1.0
        oh[WT:, SEQ, :] = ktt[s].reshape(KT, 128)
        oh[WT:, SEQ + 1, :] = 1.0
        oh_dev = np.ascontiguousarray(
            oh.reshape(NT, NCHUNK, 128, 128).transpose(0, 2, 1, 3)
            .reshape(NT * 128, NCHUNK * 128).astype(BF))

        in_maps.append({
            "word_emb": word_emb,
            "ident": np.ascontiguousarray(np.eye(128, dtype=np.float32).astype(BF)),
            "entity_vec": entity_vec,
            "postab": postab,
            "kw": kw,
            "gbb2": gbb2,
            "gbbk": gbbk,
            "oh": oh_dev,
            "idx": np.ascontiguousarray(idx),
            "kvalid": np.ascontiguousarray(kvf[s].reshape(KT, 128).T),
        })
    return in_maps


def run(inputs, trace=False):
    """Returns (full_output [64,320,768] f32, exec_time_ns or None)."""
    nc = build_program()
    in_maps = _prepare_in_maps(inputs)
    res = run_bass_kernel_spmd(nc, in_maps, list(range(NCORES)), trace=trace)
    out = np.concatenate(
        [r["out"].astype(np.float32).reshape(ROWS, SEQ, HID)
         for r in res.results], axis=0)
    return out, res.exec_time_ns


def kernel(**inputs) -> np.ndarray:
    out, _ = run(inputs)
    return out
